# revision 27
# baseline (speedup 1.0000x reference)
"""Trainium2 Bass kernel for nn_Attention2D: 2D attention over spatial axis.

Reference computation (per batch element b):
  qkv = w_qkv @ x          (1x1 conv == channel GEMM), x: [256, 4096]
  q,k,v: [8 heads, 64, 4096];  q *= 64**-0.5
  sim[h,i,j] = sum_n q[h,i,n] k[h,j,n]   (contraction over SPATIAL n=4096)
  attn = softmax(sim, axis=j)
  out[h,i,n] = sum_j attn[h,i,j] v[h,j,n]
  y = w_out @ out + b_out

Sharding: data-parallel over batch, 16 elems / 8 cores = 2 per core.

Algebraic restructuring (the attention contracts over n, so everything
factors through the 256x256 Gram matrix):
  G    = X @ X.T                      [256,256]   (537 MF)
  sim_h = Wq_h @ G @ Wk_h.T           via GqT = G @ Wq.T then tiny MMs
  attn  = softmax(sim)                (unnormalized exp; 1/Z folded later)
  M    = sum_h Wout_h @ attn_h @ Wv_h [256,256]   (tiny head-space GEMMs)
  y    = M @ X + b                    (537 MF)
This is ~4x fewer FLOPs than materializing q,k,v [512,4096].

Device dataflow per batch element (fp16 matmuls, fp32 PSUM):
  - x AND xT are both provided pre-laid-out by the host (device DMA
    transposes serialize against all other DMA traffic - xbar mode).
  - G: xT-stationary MMs accumulated over 32 n-chunks (2 row tiles);
    the two elements' G streams are interleaved with the softmax chains
    so the PE stays dense while DVE/ACT work through the softmax.
  - GqT = G @ WqT (G symmetric, so it is its own lhsT).
  - sim per head-pair: 2 N=128 MMs (both heads packed in col groups).
  - softmax over free dim j; unnormalized exp written block-diagonally;
    attnT per pair via a PE matmul against the identity; Z via DVE
    reduce_sum on the exp blocks (cheaper than ACT accumulator reads).
  - AWv_h = attn_h @ Wv_h via packed row+col diagonal MMs; 1/Z applied
    per-partition in the psum->sbuf copy.
  - MT = (Wout @ AWv).T = AWv.T @ WoutT (gives M in lhsT layout directly).
  - y = M @ X via MT-stationary MMs against x tiles + per-partition bias.
"""
import numpy as np

HEADS = 8
DH = 64
DIM = 256
HIDDEN = 512
B = 16
N = 4096            # h*w = 64*64
N_CORES = 8
B_PER_CORE = B // N_CORES
NT = N // 512       # 8 moving tiles of 512
NCH = N // 128      # 32 n-chunks of 128
PAIRS = HEADS // 2  # 4 head pairs
CC = DIM // 128     # 2 channel chunks
KC = HIDDEN // 128  # 4 hidden chunks

_nc_cache = {}


def _build():
    if "nc" in _nc_cache:
        return _nc_cache["nc"]
    from contextlib import ExitStack
    import concourse.bacc as bacc
    import concourse.tile as tile
    from concourse import mybir

    f16 = mybir.dt.float16
    f32 = mybir.dt.float32
    Exp = mybir.ActivationFunctionType.Exp
    X = mybir.AxisListType.X

    nc = bacc.Bacc("TRN2", target_bir_lowering=False, debug=False,
                   num_devices=N_CORES)
    x_d = nc.dram_tensor("x", [B_PER_CORE, DIM, N], f16, kind="ExternalInput").ap()
    xt_d = nc.dram_tensor("xt", [B_PER_CORE, 128, NCH, DIM], f16,
                          kind="ExternalInput").ap()
    wqk_d = nc.dram_tensor("wqk", [DIM, 2 * HIDDEN], f16, kind="ExternalInput").ap()
    wvn_d = nc.dram_tensor("wvn", [HIDDEN, DIM], f16, kind="ExternalInput").ap()
    wout_d = nc.dram_tensor("wout", [HIDDEN, DIM], f16, kind="ExternalInput").ap()
    b_d = nc.dram_tensor("b", [DIM], f32, kind="ExternalInput").ap()
    id_d = nc.dram_tensor("ident", [128, 128], f16, kind="ExternalInput").ap()
    y_d = nc.dram_tensor("y", [B_PER_CORE, DIM, N], f16, kind="ExternalOutput").ap()

    with tile.TileContext(nc) as tc, ExitStack() as ctx:
        consts = ctx.enter_context(tc.tile_pool(name="consts", bufs=1))
        xp = ctx.enter_context(tc.tile_pool(name="xp", bufs=2))
        xtp = ctx.enter_context(tc.tile_pool(name="xtp", bufs=2))
        midp = ctx.enter_context(tc.tile_pool(name="midp", bufs=2))
        smallp = ctx.enter_context(tc.tile_pool(name="smallp", bufs=4))
        stagep = ctx.enter_context(tc.tile_pool(name="stagep", bufs=4))
        ps8 = ctx.enter_context(tc.tile_pool(name="ps8", bufs=8, space="PSUM"))

        # ---- weights on the scalar queue (idle early); bulk data on sync ----
        wqk_t = consts.tile([128, CC, 2 * HIDDEN], f16)
        nc.scalar.dma_start(out=wqk_t[:], in_=wqk_d.rearrange("(c p) o -> p c o", p=128))
        id_t = consts.tile([128, 128], f16)
        nc.scalar.dma_start(out=id_t[:], in_=id_d)
        wvn_t = consts.tile([128, PAIRS, DIM], f16)
        nc.scalar.dma_start(out=wvn_t[:], in_=wvn_d.rearrange("(k p) o -> p k o", p=128))
        wout_t = consts.tile([128, KC, DIM], f16)
        nc.scalar.dma_start(out=wout_t[:], in_=wout_d.rearrange("(k p) o -> p k o", p=128))
        b_t = consts.tile([128, 2], f32)
        nc.scalar.dma_start(out=b_t[:], in_=b_d.rearrange("(m p) -> p m", p=128))

        # ---- HAM warmup: dummy MMs during the initial DMA wait so the PE
        #      clock gate is at 8/8 when the real work arrives ----
        ps_warm = ps8.tile([128, 512], f32, tag="ps", name="ps_warm")
        for i in range(24):
            nc.tensor.matmul(ps_warm[:], wqk_t[:, 0, 0:128], wqk_t[:, 0, 0:512],
                             start=(i == 0), stop=(i == 23))
        warm_sink = consts.tile([128, 1], f32)
        nc.vector.tensor_copy(warm_sink[:], ps_warm[:, 0:1])

        xT_ts = []
        x_ts = []
        for e in range(B_PER_CORE):
            xT_t = xtp.tile([128, NCH, DIM], f16, tag="xT", name=f"xT{e}")
            bounds = [0, 2, 4, 8, 13, 19, 26, 32] if e == 0 else [0, 8, 16, 24, 32]
            for g in range(len(bounds) - 1):
                lo, hi = bounds[g], bounds[g + 1]
                nc.sync.dma_start(out=xT_t[:, lo:hi, :], in_=xt_d[e, :, lo:hi, :])
            xT_ts.append(xT_t)
        for e in range(B_PER_CORE):
            x_t = xp.tile([128, CC, N], f16, tag="x", name=f"x{e}")
            x_src = x_d[e].rearrange("(c p) n -> p c n", p=128)
            for g in range(2):
                nc.sync.dma_start(out=x_t[:, :, g * 2048:(g + 1) * 2048],
                                  in_=x_src[:, :, g * 2048:(g + 1) * 2048])
            x_ts.append(x_t)

        # ---- G = X @ X.T (emitted per element, interleaved with chains) ----
        g_ts = {}

        g_ps = {}

        def emit_g(e, t_lo=0, t_hi=NCH):
            if e not in g_ps:
                g_ps[e] = [ps8.tile([128, DIM], f32, tag="ps", name=f"ps_g{e}_{i}")
                           for i in range(2)]
            ps_g = g_ps[e]
            for t in range(t_lo, t_hi):
                for m in range(2):
                    nc.tensor.matmul(ps_g[m][:], xT_ts[e][:, t, m * 128:(m + 1) * 128],
                                     xT_ts[e][:, t, :], start=(t == 0),
                                     stop=(t == NCH - 1))
            if t_hi == NCH:
                g_t = midp.tile([128, 2, DIM], f16, tag="g", name=f"g{e}")
                nc.vector.tensor_copy(g_t[:, 0, :], ps_g[0][:])
                nc.scalar.copy(g_t[:, 1, :], ps_g[1][:])
                g_ts[e] = g_t

        # ---- GqT (emitted per element, interleaved with the chains) ----
        gq_ts = {}

        def emit_gq(e):
            ps_gq = [ps8.tile([128, HIDDEN], f32, tag="ps", name=f"ps_gq{e}_{i}")
                     for i in range(2)]
            for m in range(2):
                for c in range(CC):
                    nc.tensor.matmul(ps_gq[m][:], g_ts[e][:, c, m * 128:(m + 1) * 128],
                                     wqk_t[:, c, 0:HIDDEN],
                                     start=(c == 0), stop=(c == CC - 1))
            gq_t = midp.tile([128, 2, HIDDEN], f16, tag="gq", name=f"gq{e}")
            nc.vector.tensor_copy(gq_t[:, 0, :], ps_gq[0][:])
            nc.scalar.copy(gq_t[:, 1, :], ps_gq[1][:])
            gq_ts[e] = gq_t

        # ---- pair chains, software-pipelined in stages so the FIFO engine
        #      queues overlap chains: A = sim+reduce, B = exp, C = consume ----
        awv_ts = [midp.tile([128, KC, DIM], f16, tag="awv", name=f"awv{e}")
                  for e in range(B_PER_CORE)]
        chains = {}

        def stage_a(e):
            gq_t = gq_ts[e]
            for p in range(PAIRS):
                ps_s = ps8.tile([128, 128], f32, tag="ps", name=f"ps_s{p}_{e}")
                co = p * 128
                for c in range(CC):
                    nc.tensor.matmul(ps_s[:], gq_t[:, c, co:co + 128],
                                     wqk_t[:, c, HIDDEN + co:HIDDEN + co + 128],
                                     start=(c == 0), stop=(c == CC - 1))
                negmax = smallp.tile([128, 1], f32, tag="negmax")
                nc.vector.reduce_max(negmax[0:64, :], ps_s[0:64, 0:64],
                                     axis=X, negate=True)
                nc.vector.reduce_max(negmax[64:128, :], ps_s[64:128, 64:128],
                                     axis=X, negate=True)
                attn_pad = smallp.tile([128, 128], f16, tag="attn_pad")
                nc.gpsimd.memset(attn_pad[0:64, 64:128], 0.0)
                nc.gpsimd.memset(attn_pad[64:128, 0:64], 0.0)
                chains[(e, p)] = (ps_s, negmax, attn_pad)

        def stage_b(e):
            for p in range(PAIRS):
                ps_s, negmax, attn_pad = chains[(e, p)]
                nc.scalar.activation(attn_pad[0:64, 0:64], ps_s[0:64, 0:64], Exp,
                                     bias=negmax[0:64, :])
                nc.scalar.activation(attn_pad[64:128, 64:128], ps_s[64:128, 64:128],
                                     Exp, bias=negmax[64:128, :])

        def stage_c1(e, act_ok):
            for p in range(PAIRS):
                _, _, attn_pad = chains[(e, p)]
                esum = smallp.tile([128, 1], f32, tag="esum")
                nc.vector.reduce_sum(esum[0:64, :], attn_pad[0:64, 0:64], axis=X)
                nc.vector.reduce_sum(esum[64:128, :], attn_pad[64:128, 64:128], axis=X)
                rz = smallp.tile([128, 1], f32, tag="rz")
                nc.vector.reciprocal(rz[:], esum[:])
                ps_t = ps8.tile([128, 128], f32, tag="ps", name="ps_t")
                nc.tensor.matmul(ps_t[:], attn_pad[:], id_t[:], start=True, stop=True)
                attnT = smallp.tile([128, 128], f16, tag="attnT")
                if act_ok and p % 2 == 1:
                    nc.scalar.copy(attnT[:], ps_t[:])
                else:
                    nc.vector.tensor_copy(attnT[:], ps_t[:])
                chains[(e, p)] = (attnT, rz)

        def stage_c2(e, act_ok):
            for p in range(PAIRS):
                attnT, rz = chains[(e, p)]
                ps_a = ps8.tile([128, DIM], f32, tag="ps")
                nc.tensor.matmul(ps_a[0:64, :], attnT[0:64, 0:64],
                                 wvn_t[0:64, p, :], start=True, stop=True)
                nc.tensor.matmul(ps_a[64:128, :], attnT[64:128, 64:128],
                                 wvn_t[64:128, p, :], start=True, stop=True)
                if act_ok and p % 2 == 1:
                    nc.scalar.mul(awv_ts[e][:, p, :], ps_a[:], rz[:])
                else:
                    nc.vector.tensor_scalar_mul(awv_ts[e][:, p, :], ps_a[:], rz[:])

        emit_g(0)
        emit_gq(0)
        stage_a(0)
        emit_g(1, 0, 16)
        stage_b(0)
        stage_c1(0, act_ok=False)
        emit_g(1, 16, NCH)
        stage_c2(0, act_ok=False)
        emit_gq(1)
        stage_a(1)

        # ---- MT + y ----
        def emit_mt(e):
            awv_t = awv_ts[e]
            ps_m = [ps8.tile([128, DIM], f32, tag="ps", name=f"ps_m{e}_{i}")
                    for i in range(2)]
            for k in range(KC):
                for m in range(2):
                    nc.tensor.matmul(ps_m[m][:], awv_t[:, k, m * 128:(m + 1) * 128],
                                     wout_t[:, k, :], start=(k == 0), stop=(k == KC - 1))
            mt_t = midp.tile([128, 2, DIM], f16, tag="mt", name=f"mt{e}")
            nc.vector.tensor_copy(mt_t[:, 0, :], ps_m[0][:])
            nc.vector.tensor_copy(mt_t[:, 1, :], ps_m[1][:])
            return mt_t

        def emit_y(e, mt_t, half=None):
            x_t = x_ts[e]
            m2s = range(2) if half is None else [half]
            for m2 in m2s:
                for t4 in range(NT // 2):
                    ps_y = [ps8.tile([128, 512], f32, tag="ps", name=f"ps_y{i}")
                            for i in range(2)]
                    for c in range(CC):
                        for h in range(2):
                            t8 = 2 * t4 + h
                            nc.tensor.matmul(
                                ps_y[h][:], mt_t[:, c, m2 * 128:(m2 + 1) * 128],
                                x_t[:, c, t8 * 512:(t8 + 1) * 512],
                                start=(c == 0), stop=(c == CC - 1))
                    y_stage = stagep.tile([128, 1024], f16, tag="y_stage")
                    nc.vector.tensor_scalar_add(y_stage[:, 0:512], ps_y[0][:],
                                                b_t[:, m2:m2 + 1])
                    nc.scalar.add(y_stage[:, 512:1024], ps_y[1][:], b_t[:, m2:m2 + 1])
                    q = nc.sync if (e + t4) % 2 == 0 else nc.scalar
                    q.dma_start(
                        out=y_d[e, m2 * 128:(m2 + 1) * 128,
                                t4 * 1024:(t4 + 1) * 1024],
                        in_=y_stage[:])

        stage_b(1)
        mt0 = emit_mt(0)
        emit_y(0, mt0, half=0)
        stage_c1(1, act_ok=True)
        stage_c2(1, act_ok=True)
        mt1 = emit_mt(1)
        emit_y(0, mt0, half=1)
        emit_y(1, mt1)

    nc.compile()
    _nc_cache["nc"] = nc
    return nc


def _prep_inputs(x, w_qkv, w_out, b_out):
    scale = DH ** (-0.5)
    wq = (w_qkv[0:HIDDEN] * scale).astype(np.float16)       # [512, 256]
    wk = w_qkv[HIDDEN:2 * HIDDEN].astype(np.float16)
    wv_nat = w_qkv[2 * HIDDEN:3 * HIDDEN].astype(np.float16).copy()  # [512, 256]
    wqk = np.concatenate([wq.T, wk.T], axis=1).copy()       # [256, 1024]
    wout_T = w_out.T.astype(np.float16).copy()              # [512, 256]
    b = b_out.astype(np.float32)
    x16 = np.ascontiguousarray(x.reshape(B, DIM, N)).astype(np.float16)
    # xT host layout [B, 128(p), 32(t), 256(c)] with n = t*128 + p
    xt16 = np.ascontiguousarray(
        x16.reshape(B, DIM, NCH, 128).transpose(0, 3, 2, 1))
    ident = np.eye(128, dtype=np.float16)
    return x16, xt16, wqk, wv_nat, wout_T, b, ident


def _run(x, w_qkv, w_out, b_out, trace=False, tmpdir=None):
    from concourse.bass_utils import run_bass_kernel_spmd

    nc = _build()
    x16, xt16, wqk, wv_nat, wout_T, b, ident = _prep_inputs(x, w_qkv, w_out, b_out)
    in_maps = [
        {"x": x16[i * B_PER_CORE:(i + 1) * B_PER_CORE],
         "xt": xt16[i * B_PER_CORE:(i + 1) * B_PER_CORE],
         "wqk": wqk, "wvn": wv_nat, "wout": wout_T, "b": b, "ident": ident}
        for i in range(N_CORES)
    ]
    kw = {}
    if trace:
        kw = {"trace": True, "tmpdir": tmpdir}
    res = run_bass_kernel_spmd(nc, in_maps, core_ids=list(range(N_CORES)), **kw)
    y = np.concatenate([res.results[i]["y"] for i in range(N_CORES)], axis=0)
    return y.astype(np.float32).reshape(B, DIM, 64, 64), res


def kernel(x, w_qkv, w_out, b_out):
    y, _ = _run(np.asarray(x), np.asarray(w_qkv), np.asarray(w_out),
                np.asarray(b_out))
    return y


# revision 28
# speedup vs baseline: 1.0273x; 1.0273x over previous
"""Trainium2 Bass kernel for nn_Attention2D: 2D attention over spatial axis.

Reference computation (per batch element b):
  qkv = w_qkv @ x          (1x1 conv == channel GEMM), x: [256, 4096]
  q,k,v: [8 heads, 64, 4096];  q *= 64**-0.5
  sim[h,i,j] = sum_n q[h,i,n] k[h,j,n]   (contraction over SPATIAL n=4096)
  attn = softmax(sim, axis=j)
  out[h,i,n] = sum_j attn[h,i,j] v[h,j,n]
  y = w_out @ out + b_out

Sharding: data-parallel over batch, 16 elems / 8 cores = 2 per core.

Algebraic restructuring (the attention contracts over n, so everything
factors through the 256x256 Gram matrix):
  G    = X @ X.T                      [256,256]   (537 MF)
  sim_h = Wq_h @ G @ Wk_h.T           via GqT = G @ Wq.T then tiny MMs
  attn  = softmax(sim)                (unnormalized exp; 1/Z folded later)
  M    = sum_h Wout_h @ attn_h @ Wv_h [256,256]   (tiny head-space GEMMs)
  y    = M @ X + b                    (537 MF)
This is ~4x fewer FLOPs than materializing q,k,v [512,4096].

Device dataflow per batch element (fp16 matmuls, fp32 PSUM):
  - x AND xT are both provided pre-laid-out by the host (device DMA
    transposes serialize against all other DMA traffic - xbar mode).
  - G: xT-stationary MMs accumulated over 32 n-chunks (2 row tiles);
    the two elements' G streams are interleaved with the softmax chains
    so the PE stays dense while DVE/ACT work through the softmax.
  - GqT = G @ WqT (G symmetric, so it is its own lhsT).
  - sim per head-pair: 2 N=128 MMs (both heads packed in col groups).
  - softmax over free dim j; unnormalized exp written block-diagonally;
    attnT per pair via a PE matmul against the identity; Z via DVE
    reduce_sum on the exp blocks (cheaper than ACT accumulator reads).
  - AWv_h = attn_h @ Wv_h via packed row+col diagonal MMs; 1/Z applied
    per-partition in the psum->sbuf copy.
  - MT = (Wout @ AWv).T = AWv.T @ WoutT (gives M in lhsT layout directly).
  - y = M @ X via MT-stationary MMs against x tiles + per-partition bias.
"""
import numpy as np

HEADS = 8
DH = 64
DIM = 256
HIDDEN = 512
B = 16
N = 4096            # h*w = 64*64
N_CORES = 8
B_PER_CORE = B // N_CORES
NT = N // 512       # 8 moving tiles of 512
NCH = N // 128      # 32 n-chunks of 128
PAIRS = HEADS // 2  # 4 head pairs
CC = DIM // 128     # 2 channel chunks
KC = HIDDEN // 128  # 4 hidden chunks

_nc_cache = {}


def _build():
    if "nc" in _nc_cache:
        return _nc_cache["nc"]
    from contextlib import ExitStack
    import concourse.bacc as bacc
    import concourse.tile as tile
    from concourse import mybir

    f16 = mybir.dt.float16
    f32 = mybir.dt.float32
    Exp = mybir.ActivationFunctionType.Exp
    X = mybir.AxisListType.X

    nc = bacc.Bacc("TRN2", target_bir_lowering=False, debug=False,
                   num_devices=N_CORES)
    x_d = nc.dram_tensor("x", [B_PER_CORE, DIM, N], f16, kind="ExternalInput").ap()
    xt_d = nc.dram_tensor("xt", [B_PER_CORE, 128, NCH, DIM], f16,
                          kind="ExternalInput").ap()
    wqk_d = nc.dram_tensor("wqk", [DIM, 2 * HIDDEN], f16, kind="ExternalInput").ap()
    wvn_d = nc.dram_tensor("wvn", [HIDDEN, DIM], f16, kind="ExternalInput").ap()
    wout_d = nc.dram_tensor("wout", [HIDDEN, DIM], f16, kind="ExternalInput").ap()
    b_d = nc.dram_tensor("b", [DIM], f32, kind="ExternalInput").ap()
    id_d = nc.dram_tensor("ident", [128, 128], f16, kind="ExternalInput").ap()
    y_d = nc.dram_tensor("y", [B_PER_CORE, DIM, N], f16, kind="ExternalOutput").ap()

    with tile.TileContext(nc) as tc, ExitStack() as ctx:
        consts = ctx.enter_context(tc.tile_pool(name="consts", bufs=1))
        xp = ctx.enter_context(tc.tile_pool(name="xp", bufs=2))
        xtp = ctx.enter_context(tc.tile_pool(name="xtp", bufs=2))
        midp = ctx.enter_context(tc.tile_pool(name="midp", bufs=2))
        smallp = ctx.enter_context(tc.tile_pool(name="smallp", bufs=4))
        stagep = ctx.enter_context(tc.tile_pool(name="stagep", bufs=4))
        ps8 = ctx.enter_context(tc.tile_pool(name="ps8", bufs=8, space="PSUM"))

        # ---- weights on the scalar queue (idle early); bulk data on sync ----
        wqk_t = consts.tile([128, CC, 2 * HIDDEN], f16)
        nc.scalar.dma_start(out=wqk_t[:], in_=wqk_d.rearrange("(c p) o -> p c o", p=128))
        id_t = consts.tile([128, 128], f16)
        nc.scalar.dma_start(out=id_t[:], in_=id_d)
        wvn_t = consts.tile([128, PAIRS, DIM], f16)
        nc.scalar.dma_start(out=wvn_t[:], in_=wvn_d.rearrange("(k p) o -> p k o", p=128))
        wout_t = consts.tile([128, KC, DIM], f16)
        nc.scalar.dma_start(out=wout_t[:], in_=wout_d.rearrange("(k p) o -> p k o", p=128))
        b_t = consts.tile([128, 2], f32)
        nc.scalar.dma_start(out=b_t[:], in_=b_d.rearrange("(m p) -> p m", p=128))

        # ---- HAM warmup: dummy MMs during the initial DMA wait so the PE
        #      clock gate is at 8/8 when the real work arrives ----
        ps_warm = ps8.tile([128, 512], f32, tag="ps", name="ps_warm")
        for i in range(10):
            nc.tensor.matmul(ps_warm[:], wqk_t[:, 0, 0:128], wqk_t[:, 0, 0:512],
                             start=(i == 0), stop=(i == 9))
        warm_sink = consts.tile([128, 1], f32)
        nc.vector.tensor_copy(warm_sink[:], ps_warm[:, 0:1])

        xT_ts = []
        x_ts = []
        for e in range(B_PER_CORE):
            xT_t = xtp.tile([128, NCH, DIM], f16, tag="xT", name=f"xT{e}")
            bounds = [0, 2, 4, 8, 13, 19, 26, 32] if e == 0 else [0, 8, 16, 24, 32]
            for g in range(len(bounds) - 1):
                lo, hi = bounds[g], bounds[g + 1]
                nc.sync.dma_start(out=xT_t[:, lo:hi, :], in_=xt_d[e, :, lo:hi, :])
            xT_ts.append(xT_t)
        for e in range(B_PER_CORE):
            x_t = xp.tile([128, CC, N], f16, tag="x", name=f"x{e}")
            x_src = x_d[e].rearrange("(c p) n -> p c n", p=128)
            for g in range(2):
                nc.sync.dma_start(out=x_t[:, :, g * 2048:(g + 1) * 2048],
                                  in_=x_src[:, :, g * 2048:(g + 1) * 2048])
            x_ts.append(x_t)

        # ---- G = X @ X.T (emitted per element, interleaved with chains) ----
        g_ts = {}

        g_ps = {}

        def emit_g(e, t_lo=0, t_hi=NCH):
            if e not in g_ps:
                g_ps[e] = [ps8.tile([128, DIM], f32, tag="ps", name=f"ps_g{e}_{i}")
                           for i in range(2)]
            ps_g = g_ps[e]
            for t in range(t_lo, t_hi):
                for m in range(2):
                    nc.tensor.matmul(ps_g[m][:], xT_ts[e][:, t, m * 128:(m + 1) * 128],
                                     xT_ts[e][:, t, :], start=(t == 0),
                                     stop=(t == NCH - 1))
            if t_hi == NCH:
                g_t = midp.tile([128, 2, DIM], f16, tag="g", name=f"g{e}")
                nc.vector.tensor_copy(g_t[:, 0, :], ps_g[0][:])
                nc.scalar.copy(g_t[:, 1, :], ps_g[1][:])
                g_ts[e] = g_t

        # ---- GqT (emitted per element, interleaved with the chains) ----
        gq_ts = {}

        def emit_gq(e):
            ps_gq = [ps8.tile([128, HIDDEN], f32, tag="ps", name=f"ps_gq{e}_{i}")
                     for i in range(2)]
            for m in range(2):
                for c in range(CC):
                    nc.tensor.matmul(ps_gq[m][:], g_ts[e][:, c, m * 128:(m + 1) * 128],
                                     wqk_t[:, c, 0:HIDDEN],
                                     start=(c == 0), stop=(c == CC - 1))
            gq_t = midp.tile([128, 2, HIDDEN], f16, tag="gq", name=f"gq{e}")
            nc.vector.tensor_copy(gq_t[:, 0, :], ps_gq[0][:])
            nc.scalar.copy(gq_t[:, 1, :], ps_gq[1][:])
            gq_ts[e] = gq_t

        # ---- pair chains, software-pipelined in stages so the FIFO engine
        #      queues overlap chains: A = sim+reduce, B = exp, C = consume ----
        awv_ts = [midp.tile([128, KC, DIM], f16, tag="awv", name=f"awv{e}")
                  for e in range(B_PER_CORE)]
        chains = {}

        def stage_a(e):
            gq_t = gq_ts[e]
            for p in range(PAIRS):
                ps_s = ps8.tile([128, 128], f32, tag="ps", name=f"ps_s{p}_{e}")
                co = p * 128
                for c in range(CC):
                    nc.tensor.matmul(ps_s[:], gq_t[:, c, co:co + 128],
                                     wqk_t[:, c, HIDDEN + co:HIDDEN + co + 128],
                                     start=(c == 0), stop=(c == CC - 1))
                negmax = smallp.tile([128, 1], f32, tag="negmax")
                nc.vector.reduce_max(negmax[0:64, :], ps_s[0:64, 0:64],
                                     axis=X, negate=True)
                nc.vector.reduce_max(negmax[64:128, :], ps_s[64:128, 64:128],
                                     axis=X, negate=True)
                attn_pad = smallp.tile([128, 128], f16, tag="attn_pad")
                nc.gpsimd.memset(attn_pad[0:64, 64:128], 0.0)
                nc.gpsimd.memset(attn_pad[64:128, 0:64], 0.0)
                chains[(e, p)] = (ps_s, negmax, attn_pad)

        def stage_b(e):
            for p in range(PAIRS):
                ps_s, negmax, attn_pad = chains[(e, p)]
                nc.scalar.activation(attn_pad[0:64, 0:64], ps_s[0:64, 0:64], Exp,
                                     bias=negmax[0:64, :])
                nc.scalar.activation(attn_pad[64:128, 64:128], ps_s[64:128, 64:128],
                                     Exp, bias=negmax[64:128, :])

        def stage_c1(e, act_ok):
            for p in range(PAIRS):
                _, _, attn_pad = chains[(e, p)]
                esum = smallp.tile([128, 1], f32, tag="esum")
                nc.vector.reduce_sum(esum[0:64, :], attn_pad[0:64, 0:64], axis=X)
                nc.vector.reduce_sum(esum[64:128, :], attn_pad[64:128, 64:128], axis=X)
                rz = smallp.tile([128, 1], f32, tag="rz")
                nc.vector.reciprocal(rz[:], esum[:])
                ps_t = ps8.tile([128, 128], f32, tag="ps", name="ps_t")
                nc.tensor.matmul(ps_t[:], attn_pad[:], id_t[:], start=True, stop=True)
                attnT = smallp.tile([128, 128], f16, tag="attnT")
                if act_ok and p % 2 == 1:
                    nc.scalar.copy(attnT[:], ps_t[:])
                else:
                    nc.vector.tensor_copy(attnT[:], ps_t[:])
                chains[(e, p)] = (attnT, rz)

        def stage_c2(e, act_ok):
            for p in range(PAIRS):
                attnT, rz = chains[(e, p)]
                ps_a = ps8.tile([128, DIM], f32, tag="ps")
                nc.tensor.matmul(ps_a[0:64, :], attnT[0:64, 0:64],
                                 wvn_t[0:64, p, :], start=True, stop=True)
                nc.tensor.matmul(ps_a[64:128, :], attnT[64:128, 64:128],
                                 wvn_t[64:128, p, :], start=True, stop=True)
                if act_ok and p % 2 == 1:
                    nc.scalar.mul(awv_ts[e][:, p, :], ps_a[:], rz[:])
                else:
                    nc.vector.tensor_scalar_mul(awv_ts[e][:, p, :], ps_a[:], rz[:])

        emit_g(0)
        emit_gq(0)
        stage_a(0)
        emit_g(1, 0, 16)
        stage_b(0)
        stage_c1(0, act_ok=False)
        emit_g(1, 16, NCH)
        stage_c2(0, act_ok=False)
        emit_gq(1)
        stage_a(1)

        # ---- MT + y ----
        def emit_mt(e):
            awv_t = awv_ts[e]
            ps_m = [ps8.tile([128, DIM], f32, tag="ps", name=f"ps_m{e}_{i}")
                    for i in range(2)]
            for k in range(KC):
                for m in range(2):
                    nc.tensor.matmul(ps_m[m][:], awv_t[:, k, m * 128:(m + 1) * 128],
                                     wout_t[:, k, :], start=(k == 0), stop=(k == KC - 1))
            mt_t = midp.tile([128, 2, DIM], f16, tag="mt", name=f"mt{e}")
            nc.vector.tensor_copy(mt_t[:, 0, :], ps_m[0][:])
            nc.vector.tensor_copy(mt_t[:, 1, :], ps_m[1][:])
            return mt_t

        def emit_y(e, mt_t, half=None):
            x_t = x_ts[e]
            m2s = range(2) if half is None else [half]
            for m2 in m2s:
                for t4 in range(NT // 2):
                    ps_y = [ps8.tile([128, 512], f32, tag="ps", name=f"ps_y{i}")
                            for i in range(2)]
                    for c in range(CC):
                        for h in range(2):
                            t8 = 2 * t4 + h
                            nc.tensor.matmul(
                                ps_y[h][:], mt_t[:, c, m2 * 128:(m2 + 1) * 128],
                                x_t[:, c, t8 * 512:(t8 + 1) * 512],
                                start=(c == 0), stop=(c == CC - 1))
                    y_stage = stagep.tile([128, 1024], f16, tag="y_stage")
                    nc.vector.tensor_scalar_add(y_stage[:, 0:512], ps_y[0][:],
                                                b_t[:, m2:m2 + 1])
                    nc.scalar.add(y_stage[:, 512:1024], ps_y[1][:], b_t[:, m2:m2 + 1])
                    q = nc.sync if (e + t4) % 2 == 0 else nc.scalar
                    q.dma_start(
                        out=y_d[e, m2 * 128:(m2 + 1) * 128,
                                t4 * 1024:(t4 + 1) * 1024],
                        in_=y_stage[:])

        stage_b(1)
        mt0 = emit_mt(0)
        emit_y(0, mt0, half=0)
        stage_c1(1, act_ok=True)
        stage_c2(1, act_ok=True)
        mt1 = emit_mt(1)
        emit_y(0, mt0, half=1)
        emit_y(1, mt1)

    nc.compile()
    _nc_cache["nc"] = nc
    return nc


def _prep_inputs(x, w_qkv, w_out, b_out):
    scale = DH ** (-0.5)
    wq = (w_qkv[0:HIDDEN] * scale).astype(np.float16)       # [512, 256]
    wk = w_qkv[HIDDEN:2 * HIDDEN].astype(np.float16)
    wv_nat = w_qkv[2 * HIDDEN:3 * HIDDEN].astype(np.float16).copy()  # [512, 256]
    wqk = np.concatenate([wq.T, wk.T], axis=1).copy()       # [256, 1024]
    wout_T = w_out.T.astype(np.float16).copy()              # [512, 256]
    b = b_out.astype(np.float32)
    x16 = np.ascontiguousarray(x.reshape(B, DIM, N)).astype(np.float16)
    # xT host layout [B, 128(p), 32(t), 256(c)] with n = t*128 + p
    xt16 = np.ascontiguousarray(
        x16.reshape(B, DIM, NCH, 128).transpose(0, 3, 2, 1))
    ident = np.eye(128, dtype=np.float16)
    return x16, xt16, wqk, wv_nat, wout_T, b, ident


def _run(x, w_qkv, w_out, b_out, trace=False, tmpdir=None):
    from concourse.bass_utils import run_bass_kernel_spmd

    nc = _build()
    x16, xt16, wqk, wv_nat, wout_T, b, ident = _prep_inputs(x, w_qkv, w_out, b_out)
    in_maps = [
        {"x": x16[i * B_PER_CORE:(i + 1) * B_PER_CORE],
         "xt": xt16[i * B_PER_CORE:(i + 1) * B_PER_CORE],
         "wqk": wqk, "wvn": wv_nat, "wout": wout_T, "b": b, "ident": ident}
        for i in range(N_CORES)
    ]
    kw = {}
    if trace:
        kw = {"trace": True, "tmpdir": tmpdir}
    res = run_bass_kernel_spmd(nc, in_maps, core_ids=list(range(N_CORES)), **kw)
    y = np.concatenate([res.results[i]["y"] for i in range(N_CORES)], axis=0)
    return y.astype(np.float32).reshape(B, DIM, 64, 64), res


def kernel(x, w_qkv, w_out, b_out):
    y, _ = _run(np.asarray(x), np.asarray(w_qkv), np.asarray(w_out),
                np.asarray(b_out))
    return y


# revision 29
# speedup vs baseline: 1.0455x; 1.0177x over previous
"""Trainium2 Bass kernel for nn_Attention2D: 2D attention over spatial axis.

Reference computation (per batch element b):
  qkv = w_qkv @ x          (1x1 conv == channel GEMM), x: [256, 4096]
  q,k,v: [8 heads, 64, 4096];  q *= 64**-0.5
  sim[h,i,j] = sum_n q[h,i,n] k[h,j,n]   (contraction over SPATIAL n=4096)
  attn = softmax(sim, axis=j)
  out[h,i,n] = sum_j attn[h,i,j] v[h,j,n]
  y = w_out @ out + b_out

Sharding: data-parallel over batch, 16 elems / 8 cores = 2 per core.

Algebraic restructuring (the attention contracts over n, so everything
factors through the 256x256 Gram matrix):
  G    = X @ X.T                      [256,256]   (537 MF)
  sim_h = Wq_h @ G @ Wk_h.T           via GqT = G @ Wq.T then tiny MMs
  attn  = softmax(sim)                (unnormalized exp; 1/Z folded later)
  M    = sum_h Wout_h @ attn_h @ Wv_h [256,256]   (tiny head-space GEMMs)
  y    = M @ X + b                    (537 MF)
This is ~4x fewer FLOPs than materializing q,k,v [512,4096].

Device dataflow per batch element (fp16 matmuls, fp32 PSUM):
  - x AND xT are both provided pre-laid-out by the host (device DMA
    transposes serialize against all other DMA traffic - xbar mode).
  - G: xT-stationary MMs accumulated over 32 n-chunks (2 row tiles);
    the two elements' G streams are interleaved with the softmax chains
    so the PE stays dense while DVE/ACT work through the softmax.
  - GqT = G @ WqT (G symmetric, so it is its own lhsT).
  - sim per head-pair: 2 N=128 MMs (both heads packed in col groups).
  - softmax over free dim j; unnormalized exp written block-diagonally;
    attnT per pair via a PE matmul against the identity; Z via DVE
    reduce_sum on the exp blocks (cheaper than ACT accumulator reads).
  - AWv_h = attn_h @ Wv_h via packed row+col diagonal MMs; 1/Z applied
    per-partition in the psum->sbuf copy.
  - MT = (Wout @ AWv).T = AWv.T @ WoutT (gives M in lhsT layout directly).
  - y = M @ X via MT-stationary MMs against x tiles + per-partition bias.
"""
import numpy as np

HEADS = 8
DH = 64
DIM = 256
HIDDEN = 512
B = 16
N = 4096            # h*w = 64*64
N_CORES = 8
B_PER_CORE = B // N_CORES
NT = N // 512       # 8 moving tiles of 512
NCH = N // 128      # 32 n-chunks of 128
PAIRS = HEADS // 2  # 4 head pairs
CC = DIM // 128     # 2 channel chunks
KC = HIDDEN // 128  # 4 hidden chunks

_nc_cache = {}


def _build():
    if "nc" in _nc_cache:
        return _nc_cache["nc"]
    from contextlib import ExitStack
    import concourse.bacc as bacc
    import concourse.tile as tile
    from concourse import mybir

    f16 = mybir.dt.float16
    f32 = mybir.dt.float32
    Exp = mybir.ActivationFunctionType.Exp
    X = mybir.AxisListType.X

    nc = bacc.Bacc("TRN2", target_bir_lowering=False, debug=False,
                   num_devices=N_CORES)
    x_d = nc.dram_tensor("x", [B_PER_CORE, DIM, N], f16, kind="ExternalInput").ap()
    xt_d = nc.dram_tensor("xt", [B_PER_CORE, 128, NCH, DIM], f16,
                          kind="ExternalInput").ap()
    wqk_d = nc.dram_tensor("wqk", [DIM, 2 * HIDDEN], f16, kind="ExternalInput").ap()
    wvn_d = nc.dram_tensor("wvn", [HIDDEN, DIM], f16, kind="ExternalInput").ap()
    wout_d = nc.dram_tensor("wout", [HIDDEN, DIM], f16, kind="ExternalInput").ap()
    b_d = nc.dram_tensor("b", [DIM], f32, kind="ExternalInput").ap()
    id_d = nc.dram_tensor("ident", [128, 128], f16, kind="ExternalInput").ap()
    y_d = nc.dram_tensor("y", [B_PER_CORE, DIM, N], f16, kind="ExternalOutput").ap()

    with tile.TileContext(nc) as tc, ExitStack() as ctx:
        consts = ctx.enter_context(tc.tile_pool(name="consts", bufs=1))
        xp = ctx.enter_context(tc.tile_pool(name="xp", bufs=2))
        xtp = ctx.enter_context(tc.tile_pool(name="xtp", bufs=2))
        midp = ctx.enter_context(tc.tile_pool(name="midp", bufs=2))
        smallp = ctx.enter_context(tc.tile_pool(name="smallp", bufs=4))
        stagep = ctx.enter_context(tc.tile_pool(name="stagep", bufs=4))
        ps8 = ctx.enter_context(tc.tile_pool(name="ps8", bufs=8, space="PSUM"))

        # ---- weights on the scalar queue (idle early); bulk data on sync ----
        wqk_t = consts.tile([128, CC, 2 * HIDDEN], f16)
        nc.scalar.dma_start(out=wqk_t[:], in_=wqk_d.rearrange("(c p) o -> p c o", p=128))
        id_t = consts.tile([128, 128], f16)
        nc.scalar.dma_start(out=id_t[:], in_=id_d)
        wvn_t = consts.tile([128, PAIRS, DIM], f16)
        nc.scalar.dma_start(out=wvn_t[:], in_=wvn_d.rearrange("(k p) o -> p k o", p=128))
        wout_t = consts.tile([128, KC, DIM], f16)
        nc.scalar.dma_start(out=wout_t[:], in_=wout_d.rearrange("(k p) o -> p k o", p=128))
        b_t = consts.tile([128, 2], f32)
        nc.scalar.dma_start(out=b_t[:], in_=b_d.rearrange("(m p) -> p m", p=128))

        xT_ts = []
        x_ts = []
        for e in range(B_PER_CORE):
            xT_t = xtp.tile([128, NCH, DIM], f16, tag="xT", name=f"xT{e}")
            bounds = [0, 2, 4, 8, 13, 19, 26, 32] if e == 0 else [0, 8, 16, 24, 32]
            for g in range(len(bounds) - 1):
                lo, hi = bounds[g], bounds[g + 1]
                nc.sync.dma_start(out=xT_t[:, lo:hi, :], in_=xt_d[e, :, lo:hi, :])
            xT_ts.append(xT_t)
        for e in range(B_PER_CORE):
            x_t = xp.tile([128, CC, N], f16, tag="x", name=f"x{e}")
            x_src = x_d[e].rearrange("(c p) n -> p c n", p=128)
            for g in range(2):
                nc.sync.dma_start(out=x_t[:, :, g * 2048:(g + 1) * 2048],
                                  in_=x_src[:, :, g * 2048:(g + 1) * 2048])
            x_ts.append(x_t)

        # ---- G = X @ X.T (emitted per element, interleaved with chains) ----
        g_ts = {}

        g_ps = {}

        def emit_g(e, t_lo=0, t_hi=NCH):
            if e not in g_ps:
                g_ps[e] = [ps8.tile([128, DIM], f32, tag="ps", name=f"ps_g{e}_{i}")
                           for i in range(2)]
            ps_g = g_ps[e]
            for t in range(t_lo, t_hi):
                for m in range(2):
                    nc.tensor.matmul(ps_g[m][:], xT_ts[e][:, t, m * 128:(m + 1) * 128],
                                     xT_ts[e][:, t, :], start=(t == 0),
                                     stop=(t == NCH - 1))
            if t_hi == NCH:
                g_t = midp.tile([128, 2, DIM], f16, tag="g", name=f"g{e}")
                nc.vector.tensor_copy(g_t[:, 0, :], ps_g[0][:])
                nc.scalar.copy(g_t[:, 1, :], ps_g[1][:])
                g_ts[e] = g_t

        # ---- GqT (emitted per element, interleaved with the chains) ----
        gq_ts = {}

        def emit_gq(e):
            ps_gq = [ps8.tile([128, HIDDEN], f32, tag="ps", name=f"ps_gq{e}_{i}")
                     for i in range(2)]
            for m in range(2):
                for c in range(CC):
                    nc.tensor.matmul(ps_gq[m][:], g_ts[e][:, c, m * 128:(m + 1) * 128],
                                     wqk_t[:, c, 0:HIDDEN],
                                     start=(c == 0), stop=(c == CC - 1))
            gq_t = midp.tile([128, 2, HIDDEN], f16, tag="gq", name=f"gq{e}")
            nc.vector.tensor_copy(gq_t[:, 0, :], ps_gq[0][:])
            nc.scalar.copy(gq_t[:, 1, :], ps_gq[1][:])
            gq_ts[e] = gq_t

        # ---- pair chains, software-pipelined in stages so the FIFO engine
        #      queues overlap chains: A = sim+reduce, B = exp, C = consume ----
        awv_ts = [midp.tile([128, KC, DIM], f16, tag="awv", name=f"awv{e}")
                  for e in range(B_PER_CORE)]
        chains = {}

        def stage_a(e):
            gq_t = gq_ts[e]
            for p in range(PAIRS):
                ps_s = ps8.tile([128, 128], f32, tag="ps", name=f"ps_s{p}_{e}")
                co = p * 128
                for c in range(CC):
                    nc.tensor.matmul(ps_s[:], gq_t[:, c, co:co + 128],
                                     wqk_t[:, c, HIDDEN + co:HIDDEN + co + 128],
                                     start=(c == 0), stop=(c == CC - 1))
                negmax = smallp.tile([128, 1], f32, tag="negmax")
                nc.vector.reduce_max(negmax[0:64, :], ps_s[0:64, 0:64],
                                     axis=X, negate=True)
                nc.vector.reduce_max(negmax[64:128, :], ps_s[64:128, 64:128],
                                     axis=X, negate=True)
                attn_pad = smallp.tile([128, 128], f16, tag="attn_pad")
                nc.gpsimd.memset(attn_pad[0:64, 64:128], 0.0)
                nc.gpsimd.memset(attn_pad[64:128, 0:64], 0.0)
                chains[(e, p)] = (ps_s, negmax, attn_pad)

        def stage_b(e):
            for p in range(PAIRS):
                ps_s, negmax, attn_pad = chains[(e, p)]
                nc.scalar.activation(attn_pad[0:64, 0:64], ps_s[0:64, 0:64], Exp,
                                     bias=negmax[0:64, :])
                nc.scalar.activation(attn_pad[64:128, 64:128], ps_s[64:128, 64:128],
                                     Exp, bias=negmax[64:128, :])

        def stage_c1(e, act_ok):
            for p in range(PAIRS):
                _, _, attn_pad = chains[(e, p)]
                esum = smallp.tile([128, 1], f32, tag="esum")
                nc.vector.reduce_sum(esum[0:64, :], attn_pad[0:64, 0:64], axis=X)
                nc.vector.reduce_sum(esum[64:128, :], attn_pad[64:128, 64:128], axis=X)
                rz = smallp.tile([128, 1], f32, tag="rz")
                nc.vector.reciprocal(rz[:], esum[:])
                ps_t = ps8.tile([128, 128], f32, tag="ps", name="ps_t")
                nc.tensor.matmul(ps_t[:], attn_pad[:], id_t[:], start=True, stop=True)
                attnT = smallp.tile([128, 128], f16, tag="attnT")
                if act_ok and p % 2 == 1:
                    nc.scalar.copy(attnT[:], ps_t[:])
                else:
                    nc.vector.tensor_copy(attnT[:], ps_t[:])
                chains[(e, p)] = (attnT, rz)

        def stage_c2(e, act_ok):
            for p in range(PAIRS):
                attnT, rz = chains[(e, p)]
                ps_a = ps8.tile([128, DIM], f32, tag="ps")
                nc.tensor.matmul(ps_a[0:64, :], attnT[0:64, 0:64],
                                 wvn_t[0:64, p, :], start=True, stop=True)
                nc.tensor.matmul(ps_a[64:128, :], attnT[64:128, 64:128],
                                 wvn_t[64:128, p, :], start=True, stop=True)
                if act_ok and p % 2 == 1:
                    nc.scalar.mul(awv_ts[e][:, p, :], ps_a[:], rz[:])
                else:
                    nc.vector.tensor_scalar_mul(awv_ts[e][:, p, :], ps_a[:], rz[:])

        emit_g(0)
        emit_gq(0)
        stage_a(0)
        emit_g(1, 0, 16)
        stage_b(0)
        stage_c1(0, act_ok=False)
        emit_g(1, 16, NCH)
        stage_c2(0, act_ok=False)
        emit_gq(1)
        stage_a(1)

        # ---- MT + y ----
        def emit_mt(e):
            awv_t = awv_ts[e]
            ps_m = [ps8.tile([128, DIM], f32, tag="ps", name=f"ps_m{e}_{i}")
                    for i in range(2)]
            for k in range(KC):
                for m in range(2):
                    nc.tensor.matmul(ps_m[m][:], awv_t[:, k, m * 128:(m + 1) * 128],
                                     wout_t[:, k, :], start=(k == 0), stop=(k == KC - 1))
            mt_t = midp.tile([128, 2, DIM], f16, tag="mt", name=f"mt{e}")
            nc.vector.tensor_copy(mt_t[:, 0, :], ps_m[0][:])
            nc.vector.tensor_copy(mt_t[:, 1, :], ps_m[1][:])
            return mt_t

        def emit_y(e, mt_t, half=None):
            x_t = x_ts[e]
            m2s = range(2) if half is None else [half]
            for m2 in m2s:
                for t4 in range(NT // 2):
                    ps_y = [ps8.tile([128, 512], f32, tag="ps", name=f"ps_y{i}")
                            for i in range(2)]
                    for c in range(CC):
                        for h in range(2):
                            t8 = 2 * t4 + h
                            nc.tensor.matmul(
                                ps_y[h][:], mt_t[:, c, m2 * 128:(m2 + 1) * 128],
                                x_t[:, c, t8 * 512:(t8 + 1) * 512],
                                start=(c == 0), stop=(c == CC - 1))
                    y_stage = stagep.tile([128, 1024], f16, tag="y_stage")
                    nc.vector.tensor_scalar_add(y_stage[:, 0:512], ps_y[0][:],
                                                b_t[:, m2:m2 + 1])
                    nc.scalar.add(y_stage[:, 512:1024], ps_y[1][:], b_t[:, m2:m2 + 1])
                    q = nc.sync if (e + t4) % 2 == 0 else nc.scalar
                    q.dma_start(
                        out=y_d[e, m2 * 128:(m2 + 1) * 128,
                                t4 * 1024:(t4 + 1) * 1024],
                        in_=y_stage[:])

        stage_b(1)
        mt0 = emit_mt(0)
        emit_y(0, mt0, half=0)
        stage_c1(1, act_ok=True)
        stage_c2(1, act_ok=True)
        mt1 = emit_mt(1)
        emit_y(0, mt0, half=1)
        emit_y(1, mt1)

    nc.compile()
    _nc_cache["nc"] = nc
    return nc


def _prep_inputs(x, w_qkv, w_out, b_out):
    scale = DH ** (-0.5)
    wq = (w_qkv[0:HIDDEN] * scale).astype(np.float16)       # [512, 256]
    wk = w_qkv[HIDDEN:2 * HIDDEN].astype(np.float16)
    wv_nat = w_qkv[2 * HIDDEN:3 * HIDDEN].astype(np.float16).copy()  # [512, 256]
    wqk = np.concatenate([wq.T, wk.T], axis=1).copy()       # [256, 1024]
    wout_T = w_out.T.astype(np.float16).copy()              # [512, 256]
    b = b_out.astype(np.float32)
    x16 = np.ascontiguousarray(x.reshape(B, DIM, N)).astype(np.float16)
    # xT host layout [B, 128(p), 32(t), 256(c)] with n = t*128 + p
    xt16 = np.ascontiguousarray(
        x16.reshape(B, DIM, NCH, 128).transpose(0, 3, 2, 1))
    ident = np.eye(128, dtype=np.float16)
    return x16, xt16, wqk, wv_nat, wout_T, b, ident


def _run(x, w_qkv, w_out, b_out, trace=False, tmpdir=None):
    from concourse.bass_utils import run_bass_kernel_spmd

    nc = _build()
    x16, xt16, wqk, wv_nat, wout_T, b, ident = _prep_inputs(x, w_qkv, w_out, b_out)
    in_maps = [
        {"x": x16[i * B_PER_CORE:(i + 1) * B_PER_CORE],
         "xt": xt16[i * B_PER_CORE:(i + 1) * B_PER_CORE],
         "wqk": wqk, "wvn": wv_nat, "wout": wout_T, "b": b, "ident": ident}
        for i in range(N_CORES)
    ]
    kw = {}
    if trace:
        kw = {"trace": True, "tmpdir": tmpdir}
    res = run_bass_kernel_spmd(nc, in_maps, core_ids=list(range(N_CORES)), **kw)
    y = np.concatenate([res.results[i]["y"] for i in range(N_CORES)], axis=0)
    return y.astype(np.float32).reshape(B, DIM, 64, 64), res


def kernel(x, w_qkv, w_out, b_out):
    y, _ = _run(np.asarray(x), np.asarray(w_qkv), np.asarray(w_out),
                np.asarray(b_out))
    return y


# revision 30
# speedup vs baseline: 1.2742x; 1.2188x over previous
"""Trainium2 Bass kernel for nn_Attention2D: 2D attention over spatial axis.

Reference computation (per batch element b):
  qkv = w_qkv @ x          (1x1 conv == channel GEMM), x: [256, 4096]
  q,k,v: [8 heads, 64, 4096];  q *= 64**-0.5
  sim[h,i,j] = sum_n q[h,i,n] k[h,j,n]   (contraction over SPATIAL n=4096)
  attn = softmax(sim, axis=j)
  out[h,i,n] = sum_j attn[h,i,j] v[h,j,n]
  y = w_out @ out + b_out

Sharding: data-parallel over batch, 16 elems / 8 cores = 2 per core.

Algebraic restructuring (the attention contracts over n, so everything
factors through the 256x256 Gram matrix):
  G    = X @ X.T                      [256,256]   (537 MF)
  sim_h = Wq_h @ G @ Wk_h.T           via GqT = G @ Wq.T then tiny MMs
  attn  = softmax(sim)                (unnormalized exp; 1/Z folded later)
  M    = sum_h Wout_h @ attn_h @ Wv_h [256,256]   (tiny head-space GEMMs)
  y    = M @ X + b                    (537 MF)
This is ~4x fewer FLOPs than materializing q,k,v [512,4096].

Device dataflow per batch element (fp16 matmuls, fp32 PSUM):
  - x AND xT are both provided pre-laid-out by the host (device DMA
    transposes serialize against all other DMA traffic - xbar mode).
  - G: xT-stationary MMs accumulated over 32 n-chunks (2 row tiles);
    the two elements' G streams are interleaved with the softmax chains
    so the PE stays dense while DVE/ACT work through the softmax.
  - GqT = G @ WqT (G symmetric, so it is its own lhsT).
  - sim per head-pair: 2 N=128 MMs (both heads packed in col groups).
  - softmax over free dim j; unnormalized exp written block-diagonally;
    attnT per pair via a PE matmul against the identity; Z via DVE
    reduce_sum on the exp blocks (cheaper than ACT accumulator reads).
  - AWv_h = attn_h @ Wv_h via packed row+col diagonal MMs; 1/Z applied
    per-partition in the psum->sbuf copy.
  - MT = (Wout @ AWv).T = AWv.T @ WoutT (gives M in lhsT layout directly).
  - y = M @ X via MT-stationary MMs against x tiles + per-partition bias.
"""
import numpy as np

HEADS = 8
DH = 64
DIM = 256
HIDDEN = 512
B = 16
N = 4096            # h*w = 64*64
N_CORES = 8
B_PER_CORE = B // N_CORES
NT = N // 512       # 8 moving tiles of 512
NCH = N // 128      # 32 n-chunks of 128
PAIRS = HEADS // 2  # 4 head pairs
CC = DIM // 128     # 2 channel chunks
KC = HIDDEN // 128  # 4 hidden chunks

_nc_cache = {}


def _build():
    if "nc" in _nc_cache:
        return _nc_cache["nc"]
    from contextlib import ExitStack
    import concourse.bacc as bacc
    import concourse.tile as tile
    from concourse import mybir

    f16 = mybir.dt.float16
    f32 = mybir.dt.float32
    Exp = mybir.ActivationFunctionType.Exp
    X = mybir.AxisListType.X

    nc = bacc.Bacc("TRN2", target_bir_lowering=False, debug=False,
                   num_devices=N_CORES)
    x_d = nc.dram_tensor("x", [B_PER_CORE, DIM, N], f16, kind="ExternalInput").ap()
    xt_d = nc.dram_tensor("xt", [B_PER_CORE, 128, NCH, DIM], f16,
                          kind="ExternalInput").ap()
    wqk_d = nc.dram_tensor("wqk", [DIM, 2 * HIDDEN], f16, kind="ExternalInput").ap()
    wvn_d = nc.dram_tensor("wvn", [HIDDEN, DIM], f16, kind="ExternalInput").ap()
    wout_d = nc.dram_tensor("wout", [HIDDEN, DIM], f16, kind="ExternalInput").ap()
    b_d = nc.dram_tensor("b", [DIM], f32, kind="ExternalInput").ap()
    id_d = nc.dram_tensor("ident", [128, 128], f16, kind="ExternalInput").ap()
    y_d = nc.dram_tensor("y", [B_PER_CORE, DIM, N], f16, kind="ExternalOutput").ap()

    with tile.TileContext(nc) as tc, ExitStack() as ctx:
        consts = ctx.enter_context(tc.tile_pool(name="consts", bufs=1))
        xp = ctx.enter_context(tc.tile_pool(name="xp", bufs=2))
        xtp = ctx.enter_context(tc.tile_pool(name="xtp", bufs=2))
        midp = ctx.enter_context(tc.tile_pool(name="midp", bufs=2))
        smallp = ctx.enter_context(tc.tile_pool(name="smallp", bufs=4))
        stagep = ctx.enter_context(tc.tile_pool(name="stagep", bufs=4))
        ps8 = ctx.enter_context(tc.tile_pool(name="ps8", bufs=8, space="PSUM"))

        # ---- weights on the scalar queue (idle early); bulk data on sync ----
        wqk_t = consts.tile([128, CC, 2 * HIDDEN], f16)
        nc.scalar.dma_start(out=wqk_t[:], in_=wqk_d.rearrange("(c p) o -> p c o", p=128))
        id_t = consts.tile([128, 128], f16)
        nc.scalar.dma_start(out=id_t[:], in_=id_d)
        wvn_t = consts.tile([128, PAIRS, DIM], f16)
        nc.scalar.dma_start(out=wvn_t[:], in_=wvn_d.rearrange("(k p) o -> p k o", p=128))
        wout_t = consts.tile([128, KC, DIM], f16)
        nc.scalar.dma_start(out=wout_t[:], in_=wout_d.rearrange("(k p) o -> p k o", p=128))
        b_t = consts.tile([128, 2], f32)
        nc.scalar.dma_start(out=b_t[:], in_=b_d.rearrange("(m p) -> p m", p=128))

        xT_ts = []
        x_ts = []
        for e in range(B_PER_CORE):
            xT_t = xtp.tile([128, NCH, DIM], f16, tag="xT", name=f"xT{e}")
            bounds = [0, 4, 10, 18, 32] if e == 0 else [0, 8, 16, 24, 32]
            for g in range(len(bounds) - 1):
                lo, hi = bounds[g], bounds[g + 1]
                nc.sync.dma_start(out=xT_t[:, lo:hi, :], in_=xt_d[e, :, lo:hi, :])
            xT_ts.append(xT_t)
        for e in range(B_PER_CORE):
            x_t = xp.tile([128, CC, N], f16, tag="x", name=f"x{e}")
            x_src = x_d[e].rearrange("(c p) n -> p c n", p=128)
            for g in range(2):
                nc.sync.dma_start(out=x_t[:, :, g * 2048:(g + 1) * 2048],
                                  in_=x_src[:, :, g * 2048:(g + 1) * 2048])
            x_ts.append(x_t)

        # ---- G = X @ X.T (emitted per element, interleaved with chains) ----
        g_ts = {}

        g_ps = {}

        def emit_g(e, t_lo=0, t_hi=NCH):
            if e not in g_ps:
                g_ps[e] = [ps8.tile([128, DIM], f32, tag="ps", name=f"ps_g{e}_{i}")
                           for i in range(2)]
            ps_g = g_ps[e]
            for t in range(t_lo, t_hi):
                for m in range(2):
                    nc.tensor.matmul(ps_g[m][:], xT_ts[e][:, t, m * 128:(m + 1) * 128],
                                     xT_ts[e][:, t, :], start=(t == 0),
                                     stop=(t == NCH - 1))
            if t_hi == NCH:
                g_t = midp.tile([128, 2, DIM], f16, tag="g", name=f"g{e}")
                nc.vector.tensor_copy(g_t[:, 0, :], ps_g[0][:])
                nc.scalar.copy(g_t[:, 1, :], ps_g[1][:])
                g_ts[e] = g_t

        # ---- GqT (emitted per element, interleaved with the chains) ----
        gq_ts = {}

        def emit_gq(e):
            ps_gq = [ps8.tile([128, HIDDEN], f32, tag="ps", name=f"ps_gq{e}_{i}")
                     for i in range(2)]
            for m in range(2):
                for c in range(CC):
                    nc.tensor.matmul(ps_gq[m][:], g_ts[e][:, c, m * 128:(m + 1) * 128],
                                     wqk_t[:, c, 0:HIDDEN],
                                     start=(c == 0), stop=(c == CC - 1))
            gq_t = midp.tile([128, 2, HIDDEN], f16, tag="gq", name=f"gq{e}")
            nc.vector.tensor_copy(gq_t[:, 0, :], ps_gq[0][:])
            nc.scalar.copy(gq_t[:, 1, :], ps_gq[1][:])
            gq_ts[e] = gq_t

        # ---- pair chains, software-pipelined in stages so the FIFO engine
        #      queues overlap chains: A = sim+reduce, B = exp, C = consume ----
        awv_ts = [midp.tile([128, KC, DIM], f16, tag="awv", name=f"awv{e}")
                  for e in range(B_PER_CORE)]
        chains = {}

        def stage_a(e):
            gq_t = gq_ts[e]
            for p in range(PAIRS):
                ps_s = ps8.tile([128, 128], f32, tag="ps", name=f"ps_s{p}_{e}")
                co = p * 128
                for c in range(CC):
                    nc.tensor.matmul(ps_s[:], gq_t[:, c, co:co + 128],
                                     wqk_t[:, c, HIDDEN + co:HIDDEN + co + 128],
                                     start=(c == 0), stop=(c == CC - 1))
                negmax = smallp.tile([128, 1], f32, tag="negmax")
                nc.vector.reduce_max(negmax[0:64, :], ps_s[0:64, 0:64],
                                     axis=X, negate=True)
                nc.vector.reduce_max(negmax[64:128, :], ps_s[64:128, 64:128],
                                     axis=X, negate=True)
                attn_pad = smallp.tile([128, 128], f16, tag="attn_pad")
                nc.gpsimd.memset(attn_pad[0:64, 64:128], 0.0)
                nc.gpsimd.memset(attn_pad[64:128, 0:64], 0.0)
                chains[(e, p)] = (ps_s, negmax, attn_pad)

        def stage_b(e):
            for p in range(PAIRS):
                ps_s, negmax, attn_pad = chains[(e, p)]
                nc.scalar.activation(attn_pad[0:64, 0:64], ps_s[0:64, 0:64], Exp,
                                     bias=negmax[0:64, :])
                nc.scalar.activation(attn_pad[64:128, 64:128], ps_s[64:128, 64:128],
                                     Exp, bias=negmax[64:128, :])

        def stage_c1(e, act_ok):
            for p in range(PAIRS):
                _, _, attn_pad = chains[(e, p)]
                esum = smallp.tile([128, 1], f32, tag="esum")
                nc.vector.reduce_sum(esum[0:64, :], attn_pad[0:64, 0:64], axis=X)
                nc.vector.reduce_sum(esum[64:128, :], attn_pad[64:128, 64:128], axis=X)
                rz = smallp.tile([128, 1], f32, tag="rz")
                nc.vector.reciprocal(rz[:], esum[:])
                ps_t = ps8.tile([128, 128], f32, tag="ps", name="ps_t")
                nc.tensor.matmul(ps_t[:], attn_pad[:], id_t[:], start=True, stop=True)
                attnT = smallp.tile([128, 128], f16, tag="attnT")
                if act_ok and p % 2 == 1:
                    nc.scalar.copy(attnT[:], ps_t[:])
                else:
                    nc.vector.tensor_copy(attnT[:], ps_t[:])
                chains[(e, p)] = (attnT, rz)

        def stage_c2(e, act_ok):
            for p in range(PAIRS):
                attnT, rz = chains[(e, p)]
                ps_a = ps8.tile([128, DIM], f32, tag="ps")
                nc.tensor.matmul(ps_a[0:64, :], attnT[0:64, 0:64],
                                 wvn_t[0:64, p, :], start=True, stop=True)
                nc.tensor.matmul(ps_a[64:128, :], attnT[64:128, 64:128],
                                 wvn_t[64:128, p, :], start=True, stop=True)
                if act_ok and p % 2 == 1:
                    nc.scalar.mul(awv_ts[e][:, p, :], ps_a[:], rz[:])
                else:
                    nc.vector.tensor_scalar_mul(awv_ts[e][:, p, :], ps_a[:], rz[:])

        emit_g(0)
        emit_gq(0)
        stage_a(0)
        emit_g(1, 0, 16)
        stage_b(0)
        stage_c1(0, act_ok=False)
        emit_g(1, 16, NCH)
        stage_c2(0, act_ok=False)
        emit_gq(1)
        stage_a(1)

        # ---- MT + y ----
        def emit_mt(e):
            awv_t = awv_ts[e]
            ps_m = [ps8.tile([128, DIM], f32, tag="ps", name=f"ps_m{e}_{i}")
                    for i in range(2)]
            for k in range(KC):
                for m in range(2):
                    nc.tensor.matmul(ps_m[m][:], awv_t[:, k, m * 128:(m + 1) * 128],
                                     wout_t[:, k, :], start=(k == 0), stop=(k == KC - 1))
            mt_t = midp.tile([128, 2, DIM], f16, tag="mt", name=f"mt{e}")
            nc.vector.tensor_copy(mt_t[:, 0, :], ps_m[0][:])
            nc.vector.tensor_copy(mt_t[:, 1, :], ps_m[1][:])
            return mt_t

        def emit_y(e, mt_t, half=None):
            x_t = x_ts[e]
            m2s = range(2) if half is None else [half]
            for m2 in m2s:
                for t4 in range(NT // 2):
                    ps_y = [ps8.tile([128, 512], f32, tag="ps", name=f"ps_y{i}")
                            for i in range(2)]
                    for c in range(CC):
                        for h in range(2):
                            t8 = 2 * t4 + h
                            nc.tensor.matmul(
                                ps_y[h][:], mt_t[:, c, m2 * 128:(m2 + 1) * 128],
                                x_t[:, c, t8 * 512:(t8 + 1) * 512],
                                start=(c == 0), stop=(c == CC - 1))
                    y_stage = stagep.tile([128, 1024], f16, tag="y_stage")
                    nc.vector.tensor_scalar_add(y_stage[:, 0:512], ps_y[0][:],
                                                b_t[:, m2:m2 + 1])
                    nc.scalar.add(y_stage[:, 512:1024], ps_y[1][:], b_t[:, m2:m2 + 1])
                    q = nc.sync if (e + t4) % 2 == 0 else nc.scalar
                    q.dma_start(
                        out=y_d[e, m2 * 128:(m2 + 1) * 128,
                                t4 * 1024:(t4 + 1) * 1024],
                        in_=y_stage[:])

        stage_b(1)
        mt0 = emit_mt(0)
        emit_y(0, mt0, half=0)
        stage_c1(1, act_ok=True)
        stage_c2(1, act_ok=True)
        mt1 = emit_mt(1)
        emit_y(0, mt0, half=1)
        emit_y(1, mt1)

    nc.compile()
    _nc_cache["nc"] = nc
    return nc


def _prep_inputs(x, w_qkv, w_out, b_out):
    scale = DH ** (-0.5)
    wq = (w_qkv[0:HIDDEN] * scale).astype(np.float16)       # [512, 256]
    wk = w_qkv[HIDDEN:2 * HIDDEN].astype(np.float16)
    wv_nat = w_qkv[2 * HIDDEN:3 * HIDDEN].astype(np.float16).copy()  # [512, 256]
    wqk = np.concatenate([wq.T, wk.T], axis=1).copy()       # [256, 1024]
    wout_T = w_out.T.astype(np.float16).copy()              # [512, 256]
    b = b_out.astype(np.float32)
    x16 = np.ascontiguousarray(x.reshape(B, DIM, N)).astype(np.float16)
    # xT host layout [B, 128(p), 32(t), 256(c)] with n = t*128 + p
    xt16 = np.ascontiguousarray(
        x16.reshape(B, DIM, NCH, 128).transpose(0, 3, 2, 1))
    ident = np.eye(128, dtype=np.float16)
    return x16, xt16, wqk, wv_nat, wout_T, b, ident


def _run(x, w_qkv, w_out, b_out, trace=False, tmpdir=None):
    from concourse.bass_utils import run_bass_kernel_spmd

    nc = _build()
    x16, xt16, wqk, wv_nat, wout_T, b, ident = _prep_inputs(x, w_qkv, w_out, b_out)
    in_maps = [
        {"x": x16[i * B_PER_CORE:(i + 1) * B_PER_CORE],
         "xt": xt16[i * B_PER_CORE:(i + 1) * B_PER_CORE],
         "wqk": wqk, "wvn": wv_nat, "wout": wout_T, "b": b, "ident": ident}
        for i in range(N_CORES)
    ]
    kw = {}
    if trace:
        kw = {"trace": True, "tmpdir": tmpdir}
    res = run_bass_kernel_spmd(nc, in_maps, core_ids=list(range(N_CORES)), **kw)
    y = np.concatenate([res.results[i]["y"] for i in range(N_CORES)], axis=0)
    return y.astype(np.float32).reshape(B, DIM, 64, 64), res


def kernel(x, w_qkv, w_out, b_out):
    y, _ = _run(np.asarray(x), np.asarray(w_qkv), np.asarray(w_out),
                np.asarray(b_out))
    return y
